# revision 17
# baseline (speedup 1.0000x reference)
"""Trainium2 Bass kernel for the latent-query attention module.

Module math (fp32 inputs):
  Q = latent @ Wq.T; K = data @ Wk.T; V = data @ Wv.T
  S = (Q K^T)/sqrt(D); P = softmax_keys(S); out = (P V) @ Wo.T + bo

Sharding: 8 cores = 4 batches x 2 head-groups (4 heads each). Each core
computes Q/K/V for its heads, full attention over all 4096 keys and all
512 queries, and a PARTIAL output projection attn_g @ Wo[:, g].T.
Host gather sums the two partials per batch, scales by 1/16 (Wk/Wv are
shipped x16, see below) and adds the bias.

Cost-model-aware design (graded time = concourse TimelineSim):
  - matmul cost = out free-size N x 0.417ns/row (bf16), x0.5 for fp8
    DoubleRow which also contracts 2x128 partitions per instruction.
  - K/V projections run as fp8e4m3 DoubleRow with host-side hi/lo
    error-feedback splits of data and 16x-scaled Wk/Wv (16x keeps the
    lo residues out of e4m3's subnormal range):
      K ~ dhi@whi + dlo@whi + dhi@wlo   (3 DR instr vs 4 bf16 instr)
    This gives better-than-bf16 K/V at 0.75x the PE cost.
  - Attention is law-split BY HEAD (softmax purity): heads 0,3 use ACT
    true Exp (scale=SCALE/16, bias=-1) emitting P directly as e4m3;
    their PV runs as key-block-PAIRED fp8 DoubleRow with e4m3 V:
    4x fewer PE cycles than the bf16 path. Heads 1,2 use the DVE
    Schraudolph bf16 exp (int16 bitpattern = EA*s + EB) and a plain
    bf16 PV, keeping their softmax at bf16 precision.
  - The ones column appended to V makes PV col 64 the softmax
    denominator (unscaled: the literal 1.0 is not part of the 16x V
    scale, so att = pv/den carries a clean 16x that the host removes).
  - exp is processed in key-block PAIRS ([128,1024] per instruction) to
    amortize ACT/DVE access-latency; PSUM: 2 banks S8 + 2 banks S12 +
    4 banks PV accumulators = 8.
"""

import sys

sys.path.insert(0, "/opt/trn_rl_repo")

import numpy as np

B, DS, DC = 4, 4096, 256
LS, LC = 512, 512
H, D = 8, 64
INNER, OUT_DIM = 512, 512
NCORES = 8
HPC = 4                 # heads per core
IH = HPC * D            # inner half = 256
KB = DS // 128          # 32 key blocks
NPAIR = KB // 2         # 16 key-block pairs
NCH = DS // 512         # 8 data chunks
SCALE = D ** -0.5
WSCALE = 16.0           # Wk/Wv host-side scale (hi/lo fp8 subnormal fix)

# Schraudolph exp for bf16 bit patterns: bf16bits(exp(s)) ~ EA*s + EB
# (SCALE and the 1/16 K-scale folded into EA; c=0).
EA = 128.0 * 1.4426950408889634 * SCALE / WSCALE
EB = 16256.0 - 5.59 + 0.5
# ACT heads: P = exp(s*SCALE/16 - C_ACT) in e4m3 (max ~90 << 240)
C_ACT = 1.0

DR_HEADS = (0, 3)       # ACT true-exp heads -> e4m3 DoubleRow PV
SCH_HEADS = (1, 2)      # DVE Schraudolph heads -> bf16 plain PV

_CACHE = {}


def _emit(ctx, tc, nc):
    from concourse import mybir

    f32 = mybir.dt.float32
    bf16 = mybir.dt.bfloat16
    e4 = mybir.dt.float8e4
    i16 = mybir.dt.int16
    Exp = mybir.ActivationFunctionType.Exp
    MUL = mybir.AluOpType.mult
    ADD = mybir.AluOpType.add
    DR = mybir.MatmulPerfMode.DoubleRow

    # ---- DRAM I/O ----
    latentT = nc.dram_tensor("latentT", [128, 4, LS], bf16, kind="ExternalInput").ap()
    wqT = nc.dram_tensor("wqT", [128, 4, IH], bf16, kind="ExternalInput").ap()
    dhi = nc.dram_tensor("dhi", [128, 2, DS], e4, kind="ExternalInput").ap()
    dlo = nc.dram_tensor("dlo", [128, 2, DS], e4, kind="ExternalInput").ap()
    wkhi = nc.dram_tensor("wkhi", [128, 2, IH], e4, kind="ExternalInput").ap()
    wklo = nc.dram_tensor("wklo", [128, 2, IH], e4, kind="ExternalInput").ap()
    wvhi = nc.dram_tensor("wvhi", [128, 2, IH], e4, kind="ExternalInput").ap()
    wvlo = nc.dram_tensor("wvlo", [128, 2, IH], e4, kind="ExternalInput").ap()
    woT = nc.dram_tensor("woT", [128, 2, OUT_DIM], bf16, kind="ExternalInput").ap()
    ident = nc.dram_tensor("ident", [128, 128], bf16, kind="ExternalInput").ap()
    outp = nc.dram_tensor("outp", [128, 4, OUT_DIM], bf16,
                          kind="ExternalOutput").ap()

    # ---- resident SBUF ----
    res = ctx.enter_context(tc.tile_pool(name="res", bufs=1))
    kt = res.tile([128, 2, DS], bf16, name="kt")            # K^T head-pairs
    v8 = res.tile([128, 2, NPAIR, 2, 64], e4, name="v8")    # V, DR heads
    vb = res.tile([128, KB, 2, 64], bf16, name="vb")        # V, sch heads
    qt = res.tile([128, HPC, LS], bf16, name="qt")          # Q^T zero-padded
    att = res.tile([128, 4, 2, 128], bf16, name="att")      # normalized [q,i]
    attnT = res.tile([128, 2, 4, 128], bf16, name="attnT")
    cbias = res.tile([128, 1], f32, name="cbias")           # -C_ACT for exp
    wts = ctx.enter_context(tc.tile_pool(name="wts", bufs=1))
    lat_s = wts.tile([128, 4, LS], bf16, name="lat_s")
    wq_s = wts.tile([128, 4, IH], bf16, name="wq_s")
    wkh_s = wts.tile([128, 2, IH], e4, name="wkh_s")
    wkl_s = wts.tile([128, 2, IH], e4, name="wkl_s")
    wvh_s = wts.tile([128, 2, IH], e4, name="wvh_s")
    wvl_s = wts.tile([128, 2, IH], e4, name="wvl_s")
    wo_s = wts.tile([128, 2, OUT_DIM], bf16, name="wo_s")
    id_s = wts.tile([128, 128], bf16, name="id_s")

    # input DMAs: SP streams the data chunks (issued first, in order);
    # SWDGE carries the weights; wo/id are deferred into ACT program order
    # after the qt copies so they can't delay chunk 0 / lat / wq.
    nc.gpsimd.dma_start(wq_s[:], wqT)
    nc.gpsimd.dma_start(wkh_s[:], wkhi)
    nc.gpsimd.dma_start(wkl_s[:], wklo)
    nc.gpsimd.dma_start(wvh_s[:], wvhi)
    nc.gpsimd.dma_start(wvl_s[:], wvlo)
    nc.scalar.dma_start(lat_s[:], latentT)

    # ---- PE warmup: dummy matmuls during the DMA lead-in so the cost
    # model's p-state ramp finishes before real work arrives. ----
    wu = res.tile([128, 72], bf16, name="wu")
    nc.vector.memset(wu[:], 0.0)
    nc.vector.memset(cbias[:], -C_ACT)
    with tc.tile_pool(name="wps", bufs=1, space="PSUM") as wps:
        wp = wps.tile([8, 64], f32, name="wp")
        for _ in range(60):
            nc.tensor.matmul(wp[:], wu[:, 0:8], wu[:, 8:72],
                             start=True, stop=True)
    nc.gpsimd.memset(qt[:], 0.0)

    # ---- merged pipeline: Q-proj, then 32 rounds (one per key block) of
    # S -> exp -> PV with the per-chunk K/V projections injected into the
    # round stream so DMA, projections and attention fully overlap.
    # PSUM: kp 1 + vp 1 + s8 1 + s12 2 + pv 2 + den 1 = 8 banks.
    if True:
        dstage = ctx.enter_context(tc.tile_pool(name="dstage", bufs=3))
        pt8p = ctx.enter_context(tc.tile_pool(name="pt8p", bufs=2))
        pt12p = ctx.enter_context(tc.tile_pool(name="pt12p", bufs=3))
        pvps = ctx.enter_context(tc.tile_pool(name="pvps", bufs=1,
                                              space="PSUM"))
        s8ps_ctx = tc.tile_pool(name="s8ps", bufs=1, space="PSUM")
        s8ps = s8ps_ctx.__enter__()
        s12ps_ctx = tc.tile_pool(name="s12ps", bufs=2, space="PSUM")
        s12ps = s12ps_ctx.__enter__()
        kqps_ctx = tc.tile_pool(name="kqps", bufs=1, space="PSUM")
        kqps = kqps_ctx.__enter__()
        vps_ctx = tc.tile_pool(name="vps", bufs=1, space="PSUM")
        vps = vps_ctx.__enter__()
        ones8 = res.tile([128, 2, 1], e4, name="ones8")
        onesb = res.tile([128, 1], bf16, name="onesb")
        nc.gpsimd.memset(ones8[:], 1.0)
        nc.gpsimd.memset(onesb[:], 1.0)
        # pv[0] holds heads 0,1; pv[1] holds heads 2,3 (1 bank each)
        pvt = [pvps.tile([128, 2, 4, 64], f32, name=f"pv{i}") for i in range(2)]
        dn = pvps.tile([128, 4, 4, 1], f32, name="dn")

        def pv_ap(h, qb):
            return pvt[h // 2][:, h % 2, qb, :]

        def load_chunk(ch):
            dh = dstage.tile([128, 2, 512], e4, tag="dh", name="dh")
            dl = dstage.tile([128, 2, 512], e4, tag="dl", name="dl")
            nc.sync.dma_start(dh[:], dhi[:, :, ch * 512:(ch + 1) * 512])
            nc.sync.dma_start(dl[:], dlo[:, :, ch * 512:(ch + 1) * 512])
            return dh, dl

        def k_half(ch, m, dh, dl):
            # K^T = 16*(Wk data^T): 3-term hi/lo fp8 DoubleRow, half m
            kp = kqps.tile([128, 512], f32, tag="kp", name="kp")
            mc = slice(m * 128, (m + 1) * 128)
            nc.tensor.matmul(kp[:], wkh_s[:, :, mc], dh[:],
                             start=True, stop=False, perf_mode=DR)
            nc.tensor.matmul(kp[:], wkh_s[:, :, mc], dl[:],
                             start=False, stop=False, perf_mode=DR)
            nc.tensor.matmul(kp[:], wkl_s[:, :, mc], dh[:],
                             start=False, stop=True, perf_mode=DR)
            nc.vector.tensor_copy(kt[:, m, ch * 512:(ch + 1) * 512], kp[:])

        def v_half(ch, half, dh, dl):
            # V = 16*(data Wv^T): 2 key-blocks x 3-term DR; -> pair 2ch+half
            vp = vps.tile([128, 2, IH], f32, tag="vp", name="vp")
            for i in range(2):
                ic = slice((2 * half + i) * 128, (2 * half + i + 1) * 128)
                nc.tensor.matmul(vp[:, i, :], dh[:, :, ic], wvh_s[:],
                                 start=True, stop=False, perf_mode=DR)
                nc.tensor.matmul(vp[:, i, :], dl[:, :, ic], wvh_s[:],
                                 start=False, stop=False, perf_mode=DR)
                nc.tensor.matmul(vp[:, i, :], dh[:, :, ic], wvl_s[:],
                                 start=False, stop=True, perf_mode=DR)
            pr = 2 * ch + half
            src = vp[:].rearrange("p j (h e) -> p j h e", e=64)
            # DR heads 0,3 -> v8 e4m3 (ACT); sch heads 1,2 -> vb bf16 (ACT)
            nc.scalar.copy(v8[:, :, pr, :, :],
                           src[:, :, 0::3, :].rearrange("p j h e -> p h j e"))
            nc.scalar.copy(vb[:, 2 * pr:2 * pr + 2, :, :], src[:, :, 1:3, :])

        def s_kb(kb, h, pool, tag):
            s_ = pool.tile([128, 512], f32, tag=tag, name=tag)
            nc.tensor.matmul(s_[:], kt[:, h // 2, kb * 128:(kb + 1) * 128],
                             qt[:, h, :], start=True, stop=True)
            return s_

        def exp8(kb, h, s_, pt):
            # pt: [128, 2, 512] e4m3 pair tile; write half kb%2
            nc.scalar.activation(pt[:, kb % 2, :], s_[:], Exp,
                                 scale=SCALE / WSCALE, bias=cbias[:])

        def exp12(kb, h, s_):
            pt = pt12p.tile([128, 512], bf16, tag=f"pt12{h}", name=f"pt12{h}")
            nc.vector.tensor_scalar(pt[:].bitcast(i16), s_[:], EA, EB,
                                    MUL, ADD)
            return pt

        def pv8(pr, h, pt):
            hid = DR_HEADS.index(h)
            first, last = pr == 0, pr == NPAIR - 1
            for qb in range(4):
                nc.tensor.matmul(
                    pv_ap(h, qb), pt[:, :, qb * 128:(qb + 1) * 128],
                    v8[:, hid, pr, :, :],
                    start=(first and qb == 0), stop=(last and qb == 3),
                    perf_mode=DR)
            for qb in range(4):
                nc.tensor.matmul(
                    dn[:, h, qb, :], pt[:, :, qb * 128:(qb + 1) * 128],
                    ones8[:], start=(first and qb == 0),
                    stop=(last and qb == 3), perf_mode=DR,
                    skip_group_check=True)

        def pv12(kb, h, pt):
            hid = SCH_HEADS.index(h)
            first, last = kb == 0, kb == KB - 1
            for qb in range(4):
                nc.tensor.matmul(
                    pv_ap(h, qb), pt[:, qb * 128:(qb + 1) * 128],
                    vb[:, kb, hid, :],
                    start=(first and qb == 0), stop=(last and qb == 3))
            for qb in range(4):
                nc.tensor.matmul(
                    dn[:, h, qb, :], pt[:, qb * 128:(qb + 1) * 128],
                    onesb[:], start=(first and qb == 0),
                    stop=(last and qb == 3), skip_group_check=True)

        # lead-in: chunk 0 + Q-projection
        drs = [load_chunk(0), load_chunk(1)]
        qp = kqps.tile([128, 512], f32, tag="kp", name="kp")
        for m in range(2):
            if m == 1:
                qp = kqps.tile([128, 512], f32, tag="kp", name="kp")
            for c in range(4):
                nc.tensor.matmul(qp[:],
                                 wq_s[:, c, m * 128:(m + 1) * 128],
                                 lat_s[:, c, :], start=(c == 0), stop=(c == 3))
            nc.scalar.copy(qt[0:64, 2 * m, :], qp[0:64, :])
            nc.scalar.copy(qt[64:128, 2 * m + 1, :], qp[64:128, :])
        # deferred input DMAs (behind qt copies in ACT program order)
        nc.scalar.dma_start(wo_s[:], woT)
        nc.scalar.dma_start(id_s[:], ident)
        k_half(0, 0, *drs[0])
        k_half(0, 1, *drs[0])
        v_half(0, 0, *drs[0])
        v_half(0, 1, *drs[0])

        # injection schedule: at round 4*(c-1)+i do piece i of chunk c
        def inject(kb):
            c = kb // 4 + 1
            if c >= NCH:
                return
            i = kb % 4
            if i == 0 and c + 1 < NCH:
                drs.append(load_chunk(c + 1))
            dh, dl = drs[c]
            if i == 0:
                k_half(c, 0, dh, dl)
            elif i == 1:
                k_half(c, 1, dh, dl)
            elif i == 2:
                v_half(c, 0, dh, dl)
            else:
                v_half(c, 1, dh, dl)

        # round loop: S/exp for kb while PV of kb-1 runs on PE
        pt8cur = {}
        prev = None
        for kb in range(KB):
            emits = []
            for h in (0, 1, 3, 2):
                if h in DR_HEADS:
                    if kb % 2 == 0:
                        pt8cur[h] = pt8p.tile([128, 2, 512], e4,
                                              tag=f"pt8{h}", name=f"pt8{h}")
                    s_ = s_kb(kb, h, s8ps, "s8")
                    exp8(kb, h, s_, pt8cur[h])
                    emits.append((h, pt8cur[h]))
                else:
                    s_ = s_kb(kb, h, s12ps, "s12")
                    emits.append((h, exp12(kb, h, s_)))
                if h == 1:
                    inject(kb)
                if h == 3 and prev is not None:
                    pkb, pe_ = prev
                    for ph, ppt in pe_:
                        if ph in DR_HEADS:
                            if pkb % 2 == 1:
                                pv8(pkb // 2, ph, ppt)
                        else:
                            pv12(pkb, ph, ppt)
            prev = (kb, emits)
        pkb, pe_ = prev
        for ph, ppt in pe_:
            if ph in DR_HEADS:
                pv8(pkb // 2, ph, ppt)
            else:
                pv12(pkb, ph, ppt)

        # ---- tail: normalize -> transpose -> out-projection -> DMA ----
        vps_ctx.__exit__(None, None, None)
        kqps_ctx.__exit__(None, None, None)
        s12ps_ctx.__exit__(None, None, None)
        s8ps_ctx.__exit__(None, None, None)
        with tc.tile_pool(name="rcp", bufs=4) as rcp, \
             tc.tile_pool(name="obuf", bufs=4) as obuf, \
             tc.tile_pool(name="tps", bufs=2, space="PSUM") as tps, \
             tc.tile_pool(name="ops", bufs=2, space="PSUM") as ops:
            from concourse import mybir as mb
            Copy = mb.ActivationFunctionType.Copy
            rcs = {}

            def recip(h):
                rc = rcp.tile([128, 4, 1], f32, tag=f"rc{h}", name=f"rc{h}")
                nc.vector.reciprocal(rc[:], dn[:, h, :, :])
                rcs[h] = rc

            def norm_mul(h, qb):
                dst = att[:, qb, h // 2, (h % 2) * 64:(h % 2 + 1) * 64]
                if h % 2 == 0:
                    nc.vector.tensor_scalar(dst, pv_ap(h, qb),
                                            rcs[h][:, qb, :], None, MUL)
                else:
                    nc.scalar.activation(dst, pv_ap(h, qb), Copy,
                                         scale=rcs[h][:, qb, :])

            for h in range(HPC):
                recip(h)
            for qb in range(4):
                for h in range(HPC):
                    norm_mul(h, qb)
                for c in range(2):
                    tp = tps.tile([128, 128], bf16, tag="tp", name="tp")
                    nc.tensor.transpose(tp[:], att[:, qb, c, :], id_s[:])
                    if c == 0:
                        nc.vector.tensor_copy(attnT[:, c, qb, :], tp[:])
                    else:
                        nc.scalar.copy(attnT[:, c, qb, :], tp[:])
                op = ops.tile([128, OUT_DIM], f32, tag="op", name="op")
                for c in range(2):
                    nc.tensor.matmul(op[:], attnT[:, c, qb, :], wo_s[:, c, :],
                                     start=(c == 0), stop=(c == 1))
                ob = obuf.tile([128, OUT_DIM], bf16, tag="ob", name="ob")
                if qb % 2 == 0:
                    nc.vector.tensor_copy(ob[:], op[:])
                else:
                    nc.scalar.copy(ob[:], op[:])
                nc.sync.dma_start(outp[:, qb, :], ob[:])


def build():
    if "nc" in _CACHE:
        return _CACHE["nc"]
    from contextlib import ExitStack

    import concourse.tile as tile
    from concourse import bacc

    nc = bacc.Bacc("TRN2", target_bir_lowering=False, debug=False,
                   num_devices=NCORES)
    with tile.TileContext(nc) as tc:
        with ExitStack() as ctx:
            _emit(ctx, tc, nc)
    nc.compile()
    _CACHE["nc"] = nc
    return nc


def _pm(a, nblk):
    """[nblk*128, f] -> partition-major [128, nblk, f] (bf16)."""
    import ml_dtypes

    f = a.shape[1]
    return np.ascontiguousarray(
        a.reshape(nblk, 128, f).transpose(1, 0, 2)).astype(ml_dtypes.bfloat16)


def _pm_hilo(a, nblk):
    """[nblk*128, f] f32 -> partition-major e4m3 (hi, lo) pair."""
    import ml_dtypes

    e4 = ml_dtypes.float8_e4m3
    f = a.shape[1]
    pm = np.ascontiguousarray(
        a.reshape(nblk, 128, f).transpose(1, 0, 2)).astype(np.float32)
    hi = pm.astype(e4)
    lo = (pm - hi.astype(np.float32)).astype(e4)
    return hi, lo


def shard(inputs):
    import ml_dtypes

    data = np.asarray(inputs["data"], dtype=np.float32)
    latent = np.asarray(inputs["latent"], dtype=np.float32)
    wq = np.asarray(inputs["Wq"], dtype=np.float32)
    wk = np.asarray(inputs["Wk"], dtype=np.float32) * WSCALE
    wv = np.asarray(inputs["Wv"], dtype=np.float32) * WSCALE
    wo = np.asarray(inputs["Wo"], dtype=np.float32)

    dataT = [_pm_hilo(np.ascontiguousarray(data[b].T), 2) for b in range(B)]
    latT = [_pm(np.ascontiguousarray(latent[b].T), 4) for b in range(B)]
    idn = np.eye(128, dtype=ml_dtypes.bfloat16)

    per_g = []
    for g in range(2):
        rows = slice(g * IH, (g + 1) * IH)
        kh, kl = _pm_hilo(np.ascontiguousarray(wk[rows, :].T), 2)
        vh, vl = _pm_hilo(np.ascontiguousarray(wv[rows, :].T), 2)
        per_g.append({
            "wqT": _pm(np.ascontiguousarray(wq[rows, :].T), 4),
            "wkhi": kh, "wklo": kl, "wvhi": vh, "wvlo": vl,
            "woT": _pm(np.ascontiguousarray(wo[:, rows].T), 2),
        })

    in_maps = []
    for i in range(NCORES):
        b, g = i // 2, i % 2
        in_maps.append({
            "dhi": dataT[b][0], "dlo": dataT[b][1],
            "latentT": latT[b], "ident": idn, **per_g[g],
        })
    return in_maps


def unshard(results, bo):
    out = np.empty((B, LS, OUT_DIM), dtype=np.float32)
    for b in range(B):
        o0 = np.asarray(results[2 * b]["outp"], dtype=np.float32)
        o1 = np.asarray(results[2 * b + 1]["outp"], dtype=np.float32)
        o = ((o0 + o1) / WSCALE).reshape(128, 4, OUT_DIM).transpose(1, 0, 2)
        out[b] = o.reshape(LS, OUT_DIM) + bo
    return out


def run(inputs, trace=False):
    from concourse import bass_utils

    nc = build()
    in_maps = shard(inputs)
    res = bass_utils.run_bass_kernel_spmd(
        nc, in_maps, core_ids=list(range(NCORES)), trace=trace)
    bo = np.asarray(inputs["bo"], dtype=np.float32).reshape(OUT_DIM)
    return unshard(res.results, bo), res


def kernel(**inputs):
    return run(inputs)[0]


# revision 18
# speedup vs baseline: 1.3733x; 1.3733x over previous
"""Trainium2 Bass kernel for the latent-query attention module.

Module math (fp32 inputs):
  Q = latent @ Wq.T; K = data @ Wk.T; V = data @ Wv.T
  S = (Q K^T)/sqrt(D); P = softmax_keys(S); out = (P V) @ Wo.T + bo

Sharding: 8 cores = 4 batches x 2 head-groups (4 heads each). Each core
computes Q/K/V for its heads, full attention over all 4096 keys and all
512 queries, and a PARTIAL output projection attn_g @ Wo[:, g].T.
Host gather sums the two partials per batch, divides by 16 (Wk/Wv ship
x16, below) and adds the bias.

Cost-model-aware design (graded time = concourse TimelineSim); this is
the baseline schedule plus two pure-PE reductions:
  - K/V projections run as fp8e4m3 DoubleRow (0.5 cycles/row AND 2x128
    contraction per instruction) using host-side hi/lo error-feedback
    splits of data and of 16x-scaled Wk/Wv (16x keeps the lo residues
    out of e4m3's subnormal range):  K ~ dhi@whi + dlo@whi + dhi@wlo.
    3 DR instructions replace 4 bf16 ones at better-than-bf16 accuracy.
  - Attention stays law-split BY HEAD (softmax purity): heads 0,3 use
    ACT true Exp emitting P directly as e4m3 (scale/16, bias -1); their
    PV runs as key-block-PAIRED fp8 DoubleRow against an e4m3 copy of
    V (+ones) that the otherwise-idle GPSIMD converts SBUF->SBUF from
    the bf16 V. 4x fewer PE cycles for those heads' PV. Heads 1,2 keep
    the DVE Schraudolph bf16 exp and the bf16 65-wide PV, so their
    softmax stays at bf16 precision (rel-err ~1.1e-2 total, tol 2e-2).
  - The ones column makes PV col 64 the softmax denominator (the 1.0 is
    not Wv-scaled, so att carries a clean 16x removed on host).
"""

import sys

sys.path.insert(0, "/opt/trn_rl_repo")

import numpy as np

B, DS, DC = 4, 4096, 256
LS, LC = 512, 512
H, D = 8, 64
INNER, OUT_DIM = 512, 512
NCORES = 8
HPC = 4                 # heads per core
IH = HPC * D            # inner half = 256
KB = DS // 128          # 32 key blocks
NPAIR = KB // 2         # 16 key-block pairs
NCH = DS // 512         # 8 data chunks
SCALE = D ** -0.5
WSCALE = 16.0           # Wk/Wv host-side scale (hi/lo fp8 subnormal fix)

# Schraudolph exp for bf16 bit patterns: bf16bits(exp(s)) ~ EA*s + EB
# (SCALE and the 1/16 K-scale folded into EA).
EA = 128.0 * 1.4426950408889634 * SCALE / WSCALE
EB = 16256.0 - 5.59 + 0.5
# ACT heads: P = exp(s*SCALE/16 - C_ACT) in e4m3 (max ~90 << 240)
C_ACT = 1.0

DR_HEADS = (0, 3)       # ACT true-exp heads -> e4m3 DoubleRow PV
_CACHE = {}


def _emit(ctx, tc, nc):
    from concourse import mybir

    f32 = mybir.dt.float32
    bf16 = mybir.dt.bfloat16
    e4 = mybir.dt.float8e4
    i16 = mybir.dt.int16
    Exp = mybir.ActivationFunctionType.Exp
    MUL = mybir.AluOpType.mult
    ADD = mybir.AluOpType.add
    DR = mybir.MatmulPerfMode.DoubleRow

    # ---- DRAM I/O (partition-major; see shard()) ----
    latentT = nc.dram_tensor("latentT", [128, 4, LS], bf16, kind="ExternalInput").ap()
    wqT = nc.dram_tensor("wqT", [128, 4, IH], bf16, kind="ExternalInput").ap()
    dhi = nc.dram_tensor("dhi", [128, 2, DS], e4, kind="ExternalInput").ap()
    dlo = nc.dram_tensor("dlo", [128, 2, DS], e4, kind="ExternalInput").ap()
    wkhi = nc.dram_tensor("wkhi", [128, 2, IH], e4, kind="ExternalInput").ap()
    wklo = nc.dram_tensor("wklo", [128, 2, IH], e4, kind="ExternalInput").ap()
    wvhi = nc.dram_tensor("wvhi", [128, 2, IH], e4, kind="ExternalInput").ap()
    wvlo = nc.dram_tensor("wvlo", [128, 2, IH], e4, kind="ExternalInput").ap()
    woT = nc.dram_tensor("woT", [128, 2, OUT_DIM], bf16, kind="ExternalInput").ap()
    ident = nc.dram_tensor("ident", [128, 128], bf16, kind="ExternalInput").ap()
    outp = nc.dram_tensor("outp", [128, 4, OUT_DIM], bf16,
                          kind="ExternalOutput").ap()

    # ---- resident SBUF ----
    res = ctx.enter_context(tc.tile_pool(name="res", bufs=1))
    kt = res.tile([128, 2, DS], bf16, name="kt")        # K^T head-pairs
    v_r = res.tile([128, KB, HPC, 65], bf16, name="v")  # V + ones col
    v8 = res.tile([128, KB, 2, 65], e4, name="v8")      # e4m3 V, DR heads
    qt = res.tile([128, HPC, LS], bf16, name="qt")      # Q^T zero-padded
    att = res.tile([128, 4, 2, 128], bf16, name="att")  # normalized [q, i]
    attnT = res.tile([128, 2, 4, 128], bf16, name="attnT")
    cbias = res.tile([128, 1], f32, name="cbias")
    wts = ctx.enter_context(tc.tile_pool(name="wts", bufs=1))
    lat_s = wts.tile([128, 4, LS], bf16, name="lat_s")
    wq_s = wts.tile([128, 4, IH], bf16, name="wq_s")
    wkh_s = wts.tile([128, 2, IH], e4, name="wkh_s")
    wkl_s = wts.tile([128, 2, IH], e4, name="wkl_s")
    wvh_s = wts.tile([128, 2, IH], e4, name="wvh_s")
    wvl_s = wts.tile([128, 2, IH], e4, name="wvl_s")
    wo_s = wts.tile([128, 2, OUT_DIM], bf16, name="wo_s")
    id_s = wts.tile([128, 128], bf16, name="id_s")

    # input DMAs, spread over issuing engines so chunk 0 arrives ASAP:
    # SP: the 8 data chunk pairs (chunk 0 first); Pool (SWDGE): weights;
    # ACT: latent early, wo/id behind the qt copies.
    nc.gpsimd.dma_start(wkh_s[:], wkhi)
    nc.gpsimd.dma_start(wkl_s[:], wklo)
    nc.gpsimd.dma_start(wq_s[:], wqT)
    nc.gpsimd.dma_start(wvh_s[:], wvhi)
    nc.gpsimd.dma_start(wvl_s[:], wvlo)
    nc.scalar.dma_start(lat_s[:], latentT)
    nc.scalar.dma_start(wo_s[:], woT)
    nc.scalar.dma_start(id_s[:], ident)

    # ---- PE warmup: ~3us of dummy matmuls during the DMA lead-in so
    # the cost model's p-state ramp finishes before real work arrives. ----
    wu = res.tile([128, 72], bf16, name="wu")
    nc.vector.memset(wu[:], 0.0)
    nc.vector.memset(cbias[:], -C_ACT)
    with tc.tile_pool(name="wps", bufs=1, space="PSUM") as wps:
        wp = wps.tile([8, 64], f32, name="wp")
        for _ in range(60):
            nc.tensor.matmul(wp[:], wu[:, 0:8], wu[:, 8:72],
                             start=True, stop=True)
    nc.gpsimd.memset(qt[:], 0.0)
    nc.gpsimd.memset(v_r[:, :, :, 64:65], 1.0)

    # exp engine schedule per head-slot (GPSIMD cannot read PSUM on hw):
    # ACT true Exp -> e4m3 pair tiles for heads 0/3; DVE Schraudolph
    # -> bf16 for heads 1/2.
    EXP_ENG = [0, 1, 1, 0]
    ptp = ctx.enter_context(tc.tile_pool(name="ptp", bufs=3))
    pt8p = ctx.enter_context(tc.tile_pool(name="pt8p", bufs=2))
    pt8cur = {}

    def exp_kb(kb, h, s_ap):
        """Returns the P tile for (kb, h): bf16 per-kb tile for sch heads,
        e4m3 pair tile (allocated on even kb) for DR heads."""
        if EXP_ENG[h] == 0:
            if kb % 2 == 0:
                pt8cur[h] = pt8p.tile([128, 2, 512], e4, tag=f"pt8{h}",
                                      name=f"pt8{h}")
            pt = pt8cur[h]
            nc.scalar.activation(pt[:, kb % 2, :], s_ap, Exp,
                                 scale=SCALE / WSCALE, bias=cbias[:])
            return pt
        pt = ptp.tile([128, 512], bf16, tag=f"pt{h}", name=f"pt{h}")
        nc.vector.tensor_scalar(pt[:].bitcast(i16), s_ap, EA, EB, MUL, ADD)
        return pt

    early_pts = []

    # ---- phases 0+1: Q^T after chunk 0, K^T/V streamed over 8 chunks ----
    # vps=3 + the sEp bank: key-block 0's S+exp units run in phase-1 PE
    # slack (one per chunk), shaving the PE-bound phase 2.
    with tc.tile_pool(name="dstage", bufs=3) as dstage, \
         tc.tile_pool(name="kvps", bufs=2, space="PSUM") as kvps, \
         tc.tile_pool(name="sEp", bufs=1, space="PSUM") as sEp, \
         tc.tile_pool(name="vps", bufs=3, space="PSUM") as vps:

        def load_chunk(ch):
            dh = dstage.tile([128, 2, 512], e4, tag="dh", name="dh")
            dl = dstage.tile([128, 2, 512], e4, tag="dl", name="dl")
            nc.sync.dma_start(dh[:], dhi[:, :, ch * 512:(ch + 1) * 512])
            nc.sync.dma_start(dl[:], dlo[:, :, ch * 512:(ch + 1) * 512])
            return dh, dl

        def kv_proj(ch, dh, dl, v_first=False):
            def k_part():
                # K^T = 16*(Wk data^T): 3-term hi/lo fp8 DoubleRow
                kp = kvps.tile([128, 2, 512], f32, tag="kp", name="kp")
                for m in range(2):
                    mc = slice(m * 128, (m + 1) * 128)
                    nc.tensor.matmul(kp[:, m, :], wkh_s[:, :, mc], dh[:],
                                     start=True, stop=False, perf_mode=DR)
                    nc.tensor.matmul(kp[:, m, :], wkh_s[:, :, mc], dl[:],
                                     start=False, stop=False, perf_mode=DR)
                    nc.tensor.matmul(kp[:, m, :], wkl_s[:, :, mc], dh[:],
                                     start=False, stop=True, perf_mode=DR)
                nc.scalar.copy(kt[:, :, ch * 512:(ch + 1) * 512], kp[:])
            if not v_first:
                k_part()
            for k2 in range(2):
                # V = 16*(data Wv^T): per key-128-block 3-term DR
                vp = vps.tile([128, 2, IH], f32, tag="vp", name="vp")
                for i in range(2):
                    ic = slice((2 * k2 + i) * 128, (2 * k2 + i + 1) * 128)
                    nc.tensor.matmul(vp[:, i, :], dh[:, :, ic], wvh_s[:],
                                     start=True, stop=False, perf_mode=DR)
                    nc.tensor.matmul(vp[:, i, :], dl[:, :, ic], wvh_s[:],
                                     start=False, stop=False, perf_mode=DR)
                    nc.tensor.matmul(vp[:, i, :], dh[:, :, ic], wvl_s[:],
                                     start=False, stop=True, perf_mode=DR)
                eng = nc.scalar if (v_first and k2 == 0) else nc.vector
                dst = v_r[:, ch * 4 + 2 * k2:ch * 4 + 2 * k2 + 2, :, 0:64]
                src = vp[:].rearrange("p b (h e) -> p b h e", e=64)
                if eng is nc.scalar:
                    eng.copy(dst, src)
                else:
                    eng.tensor_copy(dst, src)
            # idle GPSIMD converts this chunk's DR-head V to e4m3 (incl
            # the ones column written by the memset above)
            nc.gpsimd.tensor_copy(v8[:, 4 * ch:4 * ch + 4, :, :],
                                  v_r[:, 4 * ch:4 * ch + 4, 0::3, :])
            if v_first:
                k_part()

        drs = [load_chunk(0), load_chunk(1), load_chunk(2)]
        kv_proj(0, *drs[0])
        kv_proj(1, *drs[1])
        drs.append(load_chunk(3))
        kv_proj(2, *drs[2])
        # Q^T projection into the zero-padded per-head copies
        qp = kvps.tile([128, 2, 512], f32, tag="kp", name="kp")
        for m in range(2):
            for c in range(4):
                nc.tensor.matmul(qp[:, m, :],
                                 wq_s[:, c, m * 128:(m + 1) * 128],
                                 lat_s[:, c, :], start=(c == 0), stop=(c == 3))
            # rows 0:64 = head 2m, rows 64:128 = head 2m+1
            nc.scalar.copy(qt[0:64, 2 * m, :], qp[0:64, m, :])
            nc.scalar.copy(qt[64:128, 2 * m + 1, :], qp[64:128, m, :])
        for ch in range(3, NCH):
            if ch + 1 < NCH:
                drs.append(load_chunk(ch + 1))
            kv_proj(ch, *drs[ch], v_first=(ch == NCH - 1))
            if 3 <= ch <= 6:
                h = ch - 3
                sE = sEp.tile([128, 512], f32, tag="se", name="se")
                nc.tensor.matmul(sE[:], kt[:, h // 2, 0:128], qt[:, h, :],
                                 start=True, stop=True)
                early_pts.append(exp_kb(0, h, sE[:]))

    # ---- phase 2: attention (S -> exp -> PV), streamed over key blocks ----
    if True:
        pvps_ctx = tc.tile_pool(name="pvps", bufs=1, space="PSUM")
        pvps = pvps_ctx.__enter__()
        sps_ctx = tc.tile_pool(name="sps", bufs=1, space="PSUM")
        sps = sps_ctx.__enter__()
        pv = [pvps.tile([128, 4, 65], f32, name=f"pv{h}") for h in range(HPC)]
        prev = None

        def emit_s(kb, h):
            s_ = sps.tile([128, 512], f32, tag=f"s{h}", name=f"s{h}")
            nc.tensor.matmul(s_[:], kt[:, h // 2, kb * 128:(kb + 1) * 128],
                             qt[:, h, :], start=True, stop=True)
            return exp_kb(kb, h, s_[:])

        def emit_pv(kb, h, pt, qbs=range(4)):
            if EXP_ENG[h] == 0:
                if kb % 2 == 0:
                    return  # pair incomplete; fires at the odd kb
                pr = kb // 2
                hid = DR_HEADS.index(h)
                for qb in qbs:
                    nc.tensor.matmul(
                        pv[h][:, qb, :], pt[:, :, qb * 128:(qb + 1) * 128],
                        v8[:, 2 * pr:2 * pr + 2, hid, :],
                        start=(pr == 0 and qb == 0),
                        stop=(pr == NPAIR - 1 and qb == 3), perf_mode=DR)
            else:
                for qb in qbs:
                    nc.tensor.matmul(
                        pv[h][:, qb, :], pt[:, qb * 128:(qb + 1) * 128],
                        v_r[:, kb, h, :],
                        start=(kb == 0 and qb == 0),
                        stop=(kb == KB - 1 and qb == 3))

        for kb in range(KB):
            if kb == KB - 1:
                # last block: DVE-exp'd heads first so the serial DVE
                # exps (which gate the tail's reciprocals) start early
                pts = [None] * HPC
                for h in (1, 0, 2, 3):
                    pts[h] = emit_s(kb, h)
                for h in range(HPC):
                    emit_pv(prev, h, prev_pts[h])
            elif kb == 0:
                pts = early_pts  # S+exp prebuilt during phase 1
                pts = [pts[0], pts[1], pts[2], pts[3]]
            else:
                pts = [emit_s(kb, 0), emit_s(kb, 1)]
                if prev is not None:
                    emit_pv(prev, 0, prev_pts[0])
                    emit_pv(prev, 1, prev_pts[1])
                pts += [emit_s(kb, 2), emit_s(kb, 3)]
                if prev is not None:
                    emit_pv(prev, 2, prev_pts[2])
                    emit_pv(prev, 3, prev_pts[3])
            prev, prev_pts = kb, pts
        # final key block (odd -> completes the last pair) in qb-major
        # order so the tail's per-qb normalize chains unlock in turn
        for qb in range(4):
            for h in range(HPC):
                emit_pv(prev, h, prev_pts[h], qbs=(qb,))

        # ---- tail, qb-major so each query block's normalize ->
        # transpose -> out-projection -> DMA chain drains ASAP ----
        # att[q, i] = pv[q, d] / den[q] (den = col 64 of each accumulator)
        sps_ctx.__exit__(None, None, None)  # free S banks for tps/ops
        with tc.tile_pool(name="rcp", bufs=4) as rcp, \
             tc.tile_pool(name="obuf", bufs=4) as obuf, \
             tc.tile_pool(name="tps", bufs=2, space="PSUM") as tps, \
             tc.tile_pool(name="ops", bufs=2, space="PSUM") as ops:
            Copy = mybir.ActivationFunctionType.Copy
            rcs = {}

            def recip(h):
                # one batched reciprocal per head over its 4 denominators
                rc = rcp.tile([128, 4, 1], f32, tag=f"rc{h}", name=f"rc{h}")
                nc.vector.reciprocal(rc[:], pv[h][:, :, 64:65])
                rcs[h] = rc

            def norm_mul(h, qb):
                dst = att[:, qb, h // 2, (h % 2) * 64:(h % 2 + 1) * 64]
                if h % 2 == 0:
                    nc.vector.tensor_scalar(dst, pv[h][:, qb, 0:64],
                                            rcs[h][:, qb, :], None, MUL)
                else:
                    nc.scalar.activation(dst, pv[h][:, qb, 0:64], Copy,
                                         scale=rcs[h][:, qb, :])

            for h in range(HPC):
                recip(h)
            for qb in range(4):
                for h in range(HPC):
                    norm_mul(h, qb)
                for c in range(2):
                    tp = tps.tile([128, 128], bf16, tag="tp", name="tp")
                    nc.tensor.transpose(tp[:], att[:, qb, c, :], id_s[:])
                    if c == 0:
                        nc.vector.tensor_copy(attnT[:, c, qb, :], tp[:])
                    else:
                        nc.scalar.copy(attnT[:, c, qb, :], tp[:])
                op = ops.tile([128, OUT_DIM], f32, tag="op", name="op")
                for c in range(2):
                    nc.tensor.matmul(op[:], attnT[:, c, qb, :], wo_s[:, c, :],
                                     start=(c == 0), stop=(c == 1))
                ob = obuf.tile([128, OUT_DIM], bf16, tag="ob", name="ob")
                if qb % 2 == 0:
                    nc.vector.tensor_copy(ob[:], op[:])
                else:
                    nc.scalar.copy(ob[:], op[:])
                nc.sync.dma_start(outp[:, qb, :], ob[:])


def build():
    if "nc" in _CACHE:
        return _CACHE["nc"]
    from contextlib import ExitStack

    import concourse.tile as tile
    from concourse import bacc

    nc = bacc.Bacc("TRN2", target_bir_lowering=False, debug=False,
                   num_devices=NCORES)
    with tile.TileContext(nc) as tc:
        with ExitStack() as ctx:
            _emit(ctx, tc, nc)
    nc.compile()
    _CACHE["nc"] = nc
    return nc


def _pm(a, nblk):
    """[nblk*128, f] -> partition-major [128, nblk, f] (bf16)."""
    import ml_dtypes

    f = a.shape[1]
    return np.ascontiguousarray(
        a.reshape(nblk, 128, f).transpose(1, 0, 2)).astype(ml_dtypes.bfloat16)


def _pm_hilo(a, nblk):
    """[nblk*128, f] f32 -> partition-major e4m3 (hi, lo) pair."""
    import ml_dtypes

    e4 = ml_dtypes.float8_e4m3
    f = a.shape[1]
    pm = np.ascontiguousarray(
        a.reshape(nblk, 128, f).transpose(1, 0, 2)).astype(np.float32)
    hi = pm.astype(e4)
    lo = (pm - hi.astype(np.float32)).astype(e4)
    return hi, lo


def shard(inputs):
    import ml_dtypes

    data = np.asarray(inputs["data"], dtype=np.float32)
    latent = np.asarray(inputs["latent"], dtype=np.float32)
    wq = np.asarray(inputs["Wq"], dtype=np.float32)
    wk = np.asarray(inputs["Wk"], dtype=np.float32) * WSCALE
    wv = np.asarray(inputs["Wv"], dtype=np.float32) * WSCALE
    wo = np.asarray(inputs["Wo"], dtype=np.float32)

    dataT = [_pm_hilo(np.ascontiguousarray(data[b].T), 2) for b in range(B)]
    latT = [_pm(np.ascontiguousarray(latent[b].T), 4) for b in range(B)]
    idn = np.eye(128, dtype=ml_dtypes.bfloat16)

    per_g = []
    for g in range(2):
        rows = slice(g * IH, (g + 1) * IH)
        kh, kl = _pm_hilo(np.ascontiguousarray(wk[rows, :].T), 2)
        vh, vl = _pm_hilo(np.ascontiguousarray(wv[rows, :].T), 2)
        per_g.append({
            "wqT": _pm(np.ascontiguousarray(wq[rows, :].T), 4),
            "wkhi": kh, "wklo": kl, "wvhi": vh, "wvlo": vl,
            "woT": _pm(np.ascontiguousarray(wo[:, rows].T), 2),
        })

    in_maps = []
    for i in range(NCORES):
        b, g = i // 2, i % 2
        in_maps.append({
            "dhi": dataT[b][0], "dlo": dataT[b][1],
            "latentT": latT[b], "ident": idn, **per_g[g],
        })
    return in_maps


def unshard(results, bo):
    out = np.empty((B, LS, OUT_DIM), dtype=np.float32)
    for b in range(B):
        o0 = np.asarray(results[2 * b]["outp"], dtype=np.float32)
        o1 = np.asarray(results[2 * b + 1]["outp"], dtype=np.float32)
        o = ((o0 + o1) / WSCALE).reshape(128, 4, OUT_DIM).transpose(1, 0, 2)
        out[b] = o.reshape(LS, OUT_DIM) + bo
    return out


def run(inputs, trace=False):
    from concourse import bass_utils

    nc = build()
    in_maps = shard(inputs)
    res = bass_utils.run_bass_kernel_spmd(
        nc, in_maps, core_ids=list(range(NCORES)), trace=trace)
    bo = np.asarray(inputs["bo"], dtype=np.float32).reshape(OUT_DIM)
    return unshard(res.results, bo), res


def kernel(**inputs):
    return run(inputs)[0]
